# revision 17
# baseline (speedup 1.0000x reference)
"""Trainium2 Bass kernel for nn_CosineLoss: mean_i(1 - output[i, targets[i]]).

Strategy (data-parallel over the batch dim, 8 cores):
  - Core c owns rows [c*1024, (c+1)*1024) of `output` ([1024, 32000] f32 shard)
    plus flat element offsets idx[i] = i*32000 + targets[i] for its rows
    (int32, laid out [128, 8] in SBUF — one offset table per core, computed
    during input sharding; descriptor address math in the SWDGE is integer,
    while on-device ALU adds go through an fp32 path that corrupts indices
    above 2^24).
  - On device: ONE indirect DMA gathers the 1024 needed f32 elements from
    HBM (4 KB instead of 131 MB), then a free-dim reduce and a 128-partition
    matmul reduce produce a single partial-sum scalar per core.
  - Host sums the 8 partials and returns 1 - total/8192 as a () f32 array.
"""

import numpy as np

from concourse import bacc, bass, mybir
import concourse.tile as tile
from concourse.bass_utils import run_bass_kernel_spmd

N = 8192
C = 32000
NCORES = 8
NL = N // NCORES  # 1024 rows per core
P = 128
F = NL // P  # 8 gathered elements per partition

_NC_CACHE = {}


def _build():
    # Bacc (not Bass): its compile() runs generate_event_semaphores, which
    # splits multi-sem waits — walrus codegen allows 1 sync wait per inst.
    nc = bacc.Bacc("TRN2")
    x = nc.dram_tensor("x", [NL, C], mybir.dt.float32, kind="ExternalInput")
    idx = nc.dram_tensor("idx", [P, F], mybir.dt.int32, kind="ExternalInput")
    partial = nc.dram_tensor("partial", [1, 1], mybir.dt.float32, kind="ExternalOutput")

    with tile.TileContext(nc) as tc:
        with (
            tc.tile_pool(name="sbuf", bufs=1) as sbuf,
            tc.tile_pool(name="psum", bufs=1, space="PSUM") as psum,
        ):
            # Warm the SWDGE ucode before the real indices arrive: the first
            # indirect DMA pays an invisible ~3us Q7 IRAM load, so issue a
            # tiny dummy gather (idx=0, 2 partitions) that only depends on a
            # gpsimd memset — its load overlaps the idx DMA + preamble.
            warm_idx = sbuf.tile([2, 1], mybir.dt.int32)
            nc.gpsimd.memset(warm_idx[:], 0)
            warm_out = sbuf.tile([2, 1], mybir.dt.float32)
            nc.gpsimd.indirect_dma_start(
                out=warm_out[:],
                out_offset=None,
                in_=x[:],
                in_offset=bass.IndirectOffsetOnAxis(ap=warm_idx[:], axis=1),
            )

            idx_t = sbuf.tile([P, F], mybir.dt.int32)
            nc.sync.dma_start(out=idx_t[:], in_=idx[:])

            # HW unrolls one descriptor per dest partition row (the offset AP's
            # free dim is ignored), so each call gathers one element into each
            # of the 128 partitions; F calls cover all 1024 rows.
            gathered = sbuf.tile([P, F], mybir.dt.float32)
            for j in range(F):
                nc.gpsimd.indirect_dma_start(
                    out=gathered[:, j : j + 1],
                    out_offset=None,
                    in_=x[:],
                    in_offset=bass.IndirectOffsetOnAxis(
                        ap=idx_t[:, j : j + 1], axis=1
                    ),
                )

            red = sbuf.tile([P, 1], mybir.dt.float32)
            nc.vector.tensor_reduce(
                out=red[:],
                in_=gathered[:],
                axis=mybir.AxisListType.X,
                op=mybir.AluOpType.add,
            )

            # partition-reduce via matmul with ones: a [1,1] result keeps the
            # output DMA to ONE descriptor ([128,1] out takes 128 tiny ones
            # whose completion sem drips in over ~4us).
            ones = sbuf.tile([P, 1], mybir.dt.float32)
            nc.vector.memset(ones[:], 1.0)
            acc = psum.tile([1, 1], mybir.dt.float32)
            nc.tensor.matmul(out=acc[:], lhsT=red[:], rhs=ones[:], start=True, stop=True)
            res = sbuf.tile([1, 1], mybir.dt.float32)
            nc.vector.tensor_copy(out=res[:], in_=acc[:])
            # SWDGE (gpsimd) for the result store: HWDGE completion sems
            # arrive as ~16 slow +1 packets (~4us); SWDGE sems are prompt.
            nc.gpsimd.dma_start(out=partial[:], in_=res[:])

    # run Bacc passes (reg alloc, event-sem splitting); run_bass_via_pjrt
    # serializes the module without calling finalize() on prebuilt modules.
    nc.compile()
    return nc


def _get_nc():
    if "nc" not in _NC_CACHE:
        _NC_CACHE["nc"] = _build()
    return _NC_CACHE["nc"]


def _shard(output, targets):
    xs = np.ascontiguousarray(
        output.reshape(NCORES, NL, C).astype(np.float32, copy=False)
    )
    flat = np.arange(NL, dtype=np.int32) * C + targets.reshape(NCORES, NL).astype(
        np.int32
    )
    return xs, np.ascontiguousarray(flat.reshape(NCORES, P, F))


def _run(output, targets, **kwargs):
    xs, idx = _shard(output, targets)
    in_maps = [{"x": xs[c], "idx": idx[c]} for c in range(NCORES)]
    return run_bass_kernel_spmd(
        _get_nc(), in_maps, core_ids=list(range(NCORES)), **kwargs
    )


def kernel(output, targets):
    res = _run(output, targets)
    total = sum(float(r["partial"][0, 0]) for r in res.results)
    return np.array(np.float32(1.0) - np.float32(total / N), dtype=np.float32)


# revision 18
# speedup vs baseline: 1.0280x; 1.0280x over previous
"""Trainium2 Bass kernel for nn_CosineLoss: mean_i(1 - output[i, targets[i]]).

Strategy (data-parallel over the batch dim, 8 cores):
  - Core c owns rows [c*1024, (c+1)*1024) of `output` ([1024, 32000] f32 shard)
    plus flat element offsets idx[i] = i*32000 + targets[i] for its rows
    (int32, laid out [128, 8] in SBUF — one offset table per core, computed
    during input sharding; descriptor address math in the SWDGE is integer,
    while on-device ALU adds go through an fp32 path that corrupts indices
    above 2^24).
  - On device: ONE indirect DMA gathers the 1024 needed f32 elements from
    HBM (4 KB instead of 131 MB), then a free-dim reduce and a 128-partition
    matmul reduce produce a single partial-sum scalar per core.
  - Host sums the 8 partials and returns 1 - total/8192 as a () f32 array.
"""

import numpy as np

from concourse import bacc, bass, mybir
import concourse.tile as tile
from concourse.bass_utils import run_bass_kernel_spmd

N = 8192
C = 32000
NCORES = 8
NL = N // NCORES  # 1024 rows per core
P = 128
F = NL // P  # 8 gathered elements per partition

_NC_CACHE = {}


def _build():
    # Bacc (not Bass): its compile() runs generate_event_semaphores, which
    # splits multi-sem waits — walrus codegen allows 1 sync wait per inst.
    nc = bacc.Bacc("TRN2")
    x = nc.dram_tensor("x", [NL, C], mybir.dt.float32, kind="ExternalInput")
    idx = nc.dram_tensor("idx", [P, F], mybir.dt.int32, kind="ExternalInput")
    partial = nc.dram_tensor("partial", [1, 1], mybir.dt.float32, kind="ExternalOutput")

    with tile.TileContext(nc) as tc:
        with (
            tc.tile_pool(name="sbuf", bufs=1) as sbuf,
            tc.tile_pool(name="psum", bufs=1, space="PSUM") as psum,
        ):
            # Warm the SWDGE ucode before the real indices arrive: the first
            # indirect DMA pays an invisible ~3us Q7 IRAM load, so issue a
            # tiny dummy gather (idx=0, 2 partitions) that only depends on a
            # gpsimd memset — its load overlaps the idx DMA + preamble.
            warm_idx = sbuf.tile([2, 1], mybir.dt.int32)
            nc.gpsimd.memset(warm_idx[:], 0)
            warm_out = sbuf.tile([2, 1], mybir.dt.float32)
            nc.gpsimd.indirect_dma_start(
                out=warm_out[:],
                out_offset=None,
                in_=x[:],
                in_offset=bass.IndirectOffsetOnAxis(ap=warm_idx[:], axis=1),
            )

            idx_t = sbuf.tile([P, F], mybir.dt.int32)
            nc.sync.dma_start(out=idx_t[:], in_=idx[:])

            # HW unrolls one descriptor per dest partition row (the offset AP's
            # free dim is ignored), so each call gathers one element into each
            # of the 128 partitions; F calls cover all 1024 rows.
            gathered = sbuf.tile([P, F], mybir.dt.float32)
            for j in range(F):
                nc.gpsimd.indirect_dma_start(
                    out=gathered[:, j : j + 1],
                    out_offset=None,
                    in_=x[:],
                    in_offset=bass.IndirectOffsetOnAxis(
                        ap=idx_t[:, j : j + 1], axis=1
                    ),
                )

            red = sbuf.tile([P, 1], mybir.dt.float32)
            nc.vector.tensor_reduce(
                out=red[:],
                in_=gathered[:],
                axis=mybir.AxisListType.X,
                op=mybir.AluOpType.add,
            )

            # partition-reduce via matmul with ones: a [1,1] result keeps the
            # output DMA to ONE descriptor ([128,1] out takes 128 tiny ones
            # whose completion sem drips in over ~4us).
            ones = sbuf.tile([P, 1], mybir.dt.float32)
            nc.vector.memset(ones[:], 1.0)
            acc = psum.tile([1, 1], mybir.dt.float32)
            nc.tensor.matmul(out=acc[:], lhsT=red[:], rhs=ones[:], start=True, stop=True)
            res = sbuf.tile([1, 1], mybir.dt.float32)
            nc.vector.tensor_copy(out=res[:], in_=acc[:])
            nc.sync.dma_start(out=partial[:], in_=res[:])

    # run Bacc passes (reg alloc, event-sem splitting); run_bass_via_pjrt
    # serializes the module without calling finalize() on prebuilt modules.
    nc.compile()
    return nc


def _get_nc():
    if "nc" not in _NC_CACHE:
        _NC_CACHE["nc"] = _build()
    return _NC_CACHE["nc"]


def _shard(output, targets):
    xs = np.ascontiguousarray(
        output.reshape(NCORES, NL, C).astype(np.float32, copy=False)
    )
    flat = np.arange(NL, dtype=np.int32) * C + targets.reshape(NCORES, NL).astype(
        np.int32
    )
    return xs, np.ascontiguousarray(flat.reshape(NCORES, P, F))


def _run(output, targets, **kwargs):
    xs, idx = _shard(output, targets)
    in_maps = [{"x": xs[c], "idx": idx[c]} for c in range(NCORES)]
    return run_bass_kernel_spmd(
        _get_nc(), in_maps, core_ids=list(range(NCORES)), **kwargs
    )


def kernel(output, targets):
    res = _run(output, targets)
    total = sum(float(r["partial"][0, 0]) for r in res.results)
    return np.array(np.float32(1.0) - np.float32(total / N), dtype=np.float32)
